# revision 19
# baseline (speedup 1.0000x reference)
"""Trainium2 Bass kernel for nn_ExtractPatchesPositionLayer.

Reference semantics: per image b, bilinear-translate the (522,522,1) padded
object by t = -positions[b] (tfa.translate: out(y,x) = img(y+py, x+px),
zero fill outside), then center-crop 5px -> (512,512,1).

Because the shift is constant per image, floor/frac of the offset give an
integer window start (A,B) into the (zero-margin-padded) image plus four
constant bilinear corner weights. The whole bilinear then collapses into two
accumulating PE matmuls per 127-row chunk:

    psum[m, j] = sum_k Bv0[k, m] * W[k, j] + sum_k Bv1[k, m] * W[k, j+1]

with banded 128x127 matrices
    Bv0 = c00*I + c10*S,  Bv1 = c01*I + c11*S
    (I[k,m] = d_{k,m}, S[k,m] = d_{k,m+1};
     c00=(1-wy)(1-wx), c10=wy(1-wx), c01=(1-wy)wx, c11=wy*wx)

The per-image window rows are fetched with gpsimd indirect (gather) DMA:
host-precomputed int32 flat element offsets, one per gathered row, so one
SPMD program serves all cores with no data-dependent immediates and no
sequencer registers. Sharding: batch 256 -> 32 images x 8 cores,
embarrassingly parallel, no communication.
"""

from dataclasses import dataclass

import numpy as np

import concourse.bacc as bacc
import concourse.bass as bass
import concourse.mybir as mybir
import concourse.tile as tile
from concourse.bass_utils import run_bass_kernel_spmd


@dataclass(frozen=True)
class Cfg:
    bpc: int      # images per core
    n: int        # output height/width
    wpad: int     # padded input height/width (with zero margin)
    chunk: int    # output rows per matmul chunk (<=127)

    @property
    def win(self):  # window width loaded per chunk
        return self.n + 1

    @property
    def chunks(self):
        out = []
        r = 0
        while r < self.n:
            nr = min(self.chunk, self.n - r)
            out.append((r, nr))
            r += nr
        return out

    @property
    def nbig(self):
        return sum(1 for _, nr in self.chunks if nr == self.chunk)

    @property
    def rem(self):  # (row0, nrows) of the non-uniform trailing chunk, if any
        r = self.chunks[self.nbig:]
        assert len(r) <= 1
        return r[0] if r else None


def build_nc(cfg: Cfg) -> bass.Bass:
    BPC, N, WPAD, WIN = cfg.bpc, cfg.n, cfg.wpad, cfg.win
    CH = cfg.chunk
    nbig = cfg.nbig
    rem = cfg.rem
    P = CH + 1
    PS = (rem[1] + 1) if rem else 1  # partitions of the remainder gather
    TOT = BPC * WPAD * WPAD
    f32 = mybir.dt.float32
    i32 = mybir.dt.int32

    nc = bacc.Bacc("TRN2", target_bir_lowering=False, debug=False)
    x_d = nc.declare_dram_parameter("x", [BPC, WPAD, WPAD], f32, isOutput=False)
    idxb_d = nc.declare_dram_parameter("idxb", [BPC, P, nbig], i32, isOutput=False)
    idxs_d = nc.declare_dram_parameter("idxs", [BPC, PS], i32, isOutput=False)
    wmat_d = nc.declare_dram_parameter("wmat", [BPC, 128, 4], f32, isOutput=False)
    dmat_d = nc.declare_dram_parameter("dmat", [128, 2 * CH], f32, isOutput=False)
    y_d = nc.declare_dram_parameter("y", [BPC, N, N], f32, isOutput=True)

    # flat [TOT, 1] view for element-granular gathers (coef = 1)
    x_flat = bass.AP(x_d, 0, [[1, TOT], [1, 1]])

    with tile.TileContext(nc) as tc:
        with (
            tc.tile_pool(name="const", bufs=1) as constp,
            tc.tile_pool(name="bmat", bufs=3) as bmatp,
            tc.tile_pool(name="win", bufs=10) as winp,
            tc.tile_pool(name="outp", bufs=3) as outp,
            tc.tile_pool(name="ps", bufs=6, space="PSUM") as psp,
        ):
            dmat_sb = constp.tile([128, 2 * CH], f32, tag="dmat")
            nc.sync.dma_start(dmat_sb[:], dmat_d[:, :])
            wmat_sb = constp.tile([128, BPC * 4], f32, tag="wmat")
            nc.sync.dma_start(
                wmat_sb[:].rearrange("p (i q) -> p i q", q=4),
                wmat_d[:, :, :].transpose([1, 0, 2]),
            )
            idxb_sb = constp.tile([P, BPC * nbig], i32, tag="idxb")
            nc.sync.dma_start(
                idxb_sb[:].rearrange("p (i c) -> p i c", c=nbig),
                idxb_d[:, :, :].transpose([1, 0, 2]),
            )
            idxs_sb = constp.tile([PS, BPC], i32, tag="idxs")
            nc.sync.dma_start(
                idxs_sb[:],
                idxs_d[:, :].transpose([1, 0]),
            )
            d0 = dmat_sb[:, 0:CH]
            d1 = dmat_sb[:, CH:2 * CH]

            for i in range(BPC):
                # per-image banded matrices Bv0, Bv1 on DVE
                b0 = bmatp.tile([128, CH], f32, tag="b0")
                b1 = bmatp.tile([128, CH], f32, tag="b1")
                t0 = bmatp.tile([128, CH], f32, tag="t0")
                t1 = bmatp.tile([128, CH], f32, tag="t1")
                c00 = wmat_sb[:, 4 * i + 0: 4 * i + 1]
                c10 = wmat_sb[:, 4 * i + 1: 4 * i + 2]
                c01 = wmat_sb[:, 4 * i + 2: 4 * i + 3]
                c11 = wmat_sb[:, 4 * i + 3: 4 * i + 4]
                nc.scalar.mul(t0[:], d1, c10)
                nc.scalar.mul(b0[:], d0, c00)
                nc.vector.tensor_add(b0[:], b0[:], t0[:])
                nc.scalar.mul(t1[:], d1, c11)
                nc.scalar.mul(b1[:], d0, c01)
                nc.vector.tensor_add(b1[:], b1[:], t1[:])

                # one gather per uniform chunk (HW indirect DMA uses one
                # index per dest partition): wt_c[p, w] = x.flat[idxb[i,p,c]+w]
                wts = []
                for c in range(nbig):
                    wt_c = winp.tile([P, WIN], f32, tag="wt")
                    nc.gpsimd.indirect_dma_start(
                        out=wt_c[:],
                        out_offset=None,
                        in_=x_flat,
                        in_offset=bass.IndirectOffsetOnAxis(
                            ap=idxb_sb[:, nbig * i + c: nbig * i + c + 1],
                            axis=0),
                    )
                    wts.append(wt_c)
                if rem:
                    wt_s = winp.tile([PS, WIN], f32, tag="wt_s")
                    nc.gpsimd.indirect_dma_start(
                        out=wt_s[:],
                        out_offset=None,
                        in_=x_flat,
                        in_offset=bass.IndirectOffsetOnAxis(
                            ap=idxs_sb[:, i: i + 1], axis=0),
                    )

                ob_big = outp.tile([CH, nbig * N], f32, tag="ob_big")
                for c in range(nbig):
                    ps = psp.tile([CH, N], f32, tag="ps")
                    rhs0 = wts[c][:P, 0:N]
                    rhs1 = wts[c][:P, 1:1 + N]
                    nc.tensor.matmul(out=ps[:], lhsT=b0[:P, :], rhs=rhs0,
                                     start=True, stop=False)
                    nc.tensor.matmul(out=ps[:], lhsT=b1[:P, :], rhs=rhs1,
                                     start=False, stop=True)
                    nc.scalar.copy(ob_big[:, c * N:(c + 1) * N], ps[:])
                # store the uniform chunks with one strided DMA:
                # y[i, c*CH + m, j] = ob_big[m, c*N + j]
                nc.sync.dma_start(
                    bass.AP(y_d, i * (N * N),
                            [[N, CH], [CH * N, nbig], [1, N]]),
                    ob_big[:].rearrange("p (c w) -> p c w", w=N),
                )
                if rem:
                    r0r, nrr = rem
                    ps_s = psp.tile([CH, N], f32, tag="ps")
                    ob_s = outp.tile([max(nrr, 1), N], f32, tag="ob_s")
                    nc.tensor.matmul(out=ps_s[:nrr, :],
                                     lhsT=b0[:nrr + 1, :nrr],
                                     rhs=wt_s[:nrr + 1, 0:N],
                                     start=True, stop=False)
                    nc.tensor.matmul(out=ps_s[:nrr, :],
                                     lhsT=b1[:nrr + 1, :nrr],
                                     rhs=wt_s[:nrr + 1, 1:N + 1],
                                     start=False, stop=True)
                    nc.scalar.copy(ob_s[:nrr, :], ps_s[:nrr, :])
                    nc.sync.dma_start(y_d[i, r0r:r0r + nrr, :], ob_s[:nrr, :])
    nc.compile()
    return nc


def host_prep(padded: np.ndarray, positions: np.ndarray, n_cores: int, chunk: int):
    """Shard + build metadata. padded: (B, npad, npad) f32, positions: (B, 2)."""
    B, npad, _ = padded.shape
    n = npad - 10
    bpc = B // n_cores
    win = n + 1

    px = positions[:, 0].astype(np.float32)
    py = positions[:, 1].astype(np.float32)
    fy = np.floor(py)
    fx = np.floor(px)
    ay = (5 + fy).astype(np.int64)
    ax = (5 + fx).astype(np.int64)
    wy = (py - fy).astype(np.float32)
    wx = (px - fx).astype(np.float32)

    m_lo = int(max(0, -min(ay.min(), ax.min())))
    m_hi = int(max(0, max(ay.max(), ax.max()) + win - npad))
    wpad = npad + m_lo + m_hi

    pp = np.zeros((B, wpad, wpad), dtype=np.float32)
    pp[:, m_lo:m_lo + npad, m_lo:m_lo + npad] = padded

    c00 = ((1 - wy) * (1 - wx)).astype(np.float32)
    c10 = (wy * (1 - wx)).astype(np.float32)
    c01 = ((1 - wy) * wx).astype(np.float32)
    c11 = (wy * wx).astype(np.float32)

    dmat = np.zeros((128, 2 * chunk), dtype=np.float32)
    for m in range(chunk):
        dmat[m, m] = 1.0            # I
        dmat[m + 1, chunk + m] = 1.0  # S

    cfg = Cfg(bpc=bpc, n=n, wpad=wpad, chunk=chunk)
    nbig = cfg.nbig
    rem = cfg.rem
    P = chunk + 1
    PS = (rem[1] + 1) if rem else 1

    in_maps = []
    for cidx in range(n_cores):
        sl = slice(cidx * bpc, (cidx + 1) * bpc)
        A = (ay[sl] + m_lo).astype(np.int64)
        Bc = (ax[sl] + m_lo).astype(np.int64)
        base = np.arange(bpc, dtype=np.int64) * (wpad * wpad)
        # idxb[i, p, c] = flat offset of row (A + c*chunk + p), col B
        pgrid = np.arange(P)[None, :, None]
        cgrid = np.arange(nbig)[None, None, :]
        idxb = (base[:, None, None]
                + (A[:, None, None] + cgrid * chunk + pgrid) * wpad
                + Bc[:, None, None]).astype(np.int32)
        if rem:
            r0r, _ = rem
            psgrid = np.arange(PS)[None, :]
            idxs = (base[:, None] + (A[:, None] + r0r + psgrid) * wpad
                    + Bc[:, None]).astype(np.int32)
        else:
            idxs = np.zeros((bpc, PS), dtype=np.int32)
        wmat = np.empty((bpc, 128, 4), dtype=np.float32)
        wmat[:, :, 0] = c00[sl][:, None]
        wmat[:, :, 1] = c10[sl][:, None]
        wmat[:, :, 2] = c01[sl][:, None]
        wmat[:, :, 3] = c11[sl][:, None]
        in_maps.append({
            "x": np.ascontiguousarray(pp[sl]),
            "idxb": idxb,
            "idxs": idxs,
            "wmat": wmat,
            "dmat": dmat,
        })
    return cfg, in_maps


N_CORES = 8
CHUNK = 127
_nc_cache: dict = {}


def kernel(padded_obj: np.ndarray, positions: np.ndarray) -> np.ndarray:
    padded_obj = np.asarray(padded_obj)
    positions = np.asarray(positions)
    B, npad, _, C = padded_obj.shape
    cfg, in_maps = host_prep(
        padded_obj.reshape(B, npad, npad).astype(np.float32, copy=False),
        positions, N_CORES, CHUNK)

    nc = _nc_cache.get(cfg)
    if nc is None:
        nc = build_nc(cfg)
        _nc_cache[cfg] = nc

    res = run_bass_kernel_spmd(nc, in_maps, core_ids=list(range(N_CORES)))
    out = np.concatenate([r["y"] for r in res.results], axis=0)
    return out.reshape(B, cfg.n, cfg.n, 1).astype(np.float32, copy=False)
